# revision 4
# baseline (speedup 1.0000x reference)
"""Trainium2 Bass kernel for nn_AttentionFusion (cross-attention, B=4, LQ=1024,
LKV=4096, D=512, H=4 heads of 128).

Sharding: 8 cores = (batch b in 0..3) x (head-pair hp in 0..1). Core c = 2*b+hp
computes attention for heads {2hp, 2hp+1} of batch b plus its partial
out-projection (tensor-parallel split of Wo rows). Host sums the two partials
per batch (the TP un-shard) — the per-head softmax and all matmuls run on
device in bf16 with fp32 accumulation.

Device layout (per core):
  xT  [e,q]   <- cast-load x + DMA-transpose      (bf16)
  eT  [e,kv]  <- cast-load encoder + DMA-transpose (bf16)
  qT/kT [d,*] <- W.T-stationary projections, per-partition bias fused in the
                 PSUM->SBUF copy (d is the partition dim in transposed layout)
  v   [kv,d]  <- encoder-stationary projection (bv folded into cvec: since
                 softmax rows sum to 1, attn@(v0+bv) = attn@v0 + bv, and
                 (bv @ Wo_slice.T) is a constant vector added at the end)
  scoresT [kv,q] = kT-tile.T @ qT   (PSUM, fp32)
  P = exp(scale*scoresT) on ACT  -> bf16 (no row-max: |scores| <~ 6)
  ctx~T [d,q] += v-tile.T @ P       (PSUM accumulate over kv tiles)
  denom via ones.T @ ACC (ACC = DVE running sum of P tiles), reciprocal,
  partition-broadcast, fused into the ctx PSUM->SBUF copy (normalize там)
  out[q,e] = ctxT.T @ Wo_slice.T + cvec  -> DRAM
"""

import numpy as np

B, LQ, LKV, D, H, HD = 4, 1024, 4096, 512, 4, 128
NCORES = 8
SCALE = 1.0 / float(np.sqrt(HD))

_compiled = {}


def _build():
    import concourse.bacc as bacc
    import concourse.mybir as mybir
    from concourse import tile

    bf16, f32 = mybir.dt.bfloat16, mybir.dt.float32
    EXP = mybir.ActivationFunctionType.Exp

    nc = bacc.Bacc(
        "TRN2",
        target_bir_lowering=False,
        debug=False,
        enable_asserts=True,
        num_devices=NCORES,
    )

    xb = nc.dram_tensor("xb", [LQ, D], f32, kind="ExternalInput")
    enc = nc.dram_tensor("enc", [LKV, D], f32, kind="ExternalInput")
    wqt = nc.dram_tensor("wqt", [D, 256], f32, kind="ExternalInput")  # Wq[sl].T
    wkt = nc.dram_tensor("wkt", [D, 256], f32, kind="ExternalInput")
    wvt = nc.dram_tensor("wvt", [D, 256], f32, kind="ExternalInput")
    wot = nc.dram_tensor("wot", [256, D], f32, kind="ExternalInput")  # Wo[:,sl].T
    bq2 = nc.dram_tensor("bq2", [128, 2], f32, kind="ExternalInput")
    bk2 = nc.dram_tensor("bk2", [128, 2], f32, kind="ExternalInput")
    cvec = nc.dram_tensor("cvec", [D], f32, kind="ExternalInput")
    outp = nc.dram_tensor("outp", [LQ, D], f32, kind="ExternalOutput")

    with tile.TileContext(nc) as tc:
        with (
            tc.tile_pool(name="const", bufs=1) as const,
            tc.tile_pool(name="big", bufs=1) as big,
            tc.tile_pool(name="esbp", bufs=2) as esbp,
            tc.tile_pool(name="expp", bufs=4) as expp,
            tc.tile_pool(name="accp", bufs=2) as accp,
            tc.tile_pool(name="rcp", bufs=2) as rcp,
            tc.tile_pool(name="osb", bufs=3) as osb,
            tc.tile_pool(name="ps_s", bufs=2, space="PSUM") as ps_s,
            tc.tile_pool(name="ps_c", bufs=1, space="PSUM") as ps_c,
            tc.tile_pool(name="ps_m", bufs=2, space="PSUM") as ps_m,
        ):
            # --- constants ---
            ones = const.tile([128, 1], f32)
            nc.vector.memset(ones[:], 1.0)
            bqsb = const.tile([128, 2], f32)
            nc.sync.dma_start(bqsb[:], bq2[:])
            bksb = const.tile([128, 2], f32)
            nc.sync.dma_start(bksb[:], bk2[:])
            cvst = const.tile([128, D], f32)
            nc.sync.dma_start(cvst[0:1, :], cvec.ap().unsqueeze(0))
            cvsb = const.tile([128, D], f32)
            nc.gpsimd.partition_broadcast(cvsb[:], cvst[0:1, :])
            # warm up the ACT exp table set early (~2.7us table load)
            warm = const.tile([128, 1], f32)
            nc.scalar.activation(warm[:], ones[:], EXP)

            # --- phase 0: loads (cast f32->bf16 on SWDGE) + DMA transposes ---
            x_sb = big.tile([128, 8, 512], bf16)
            nc.gpsimd.dma_start(x_sb[:], xb.ap().rearrange("(t p) e -> p t e", p=128))
            xT = big.tile([128, 4, LQ], bf16)
            for t in range(8):
                nc.sync.dma_start(
                    xT[:, :, 128 * t : 128 * t + 128], x_sb[:, t, :], transpose=True
                )

            wq_sb = const.tile([128, 4, 256], bf16)
            nc.gpsimd.dma_start(wq_sb[:], wqt.ap().rearrange("(k p) d -> p k d", p=128))
            wk_sb = const.tile([128, 4, 256], bf16)
            nc.gpsimd.dma_start(wk_sb[:], wkt.ap().rearrange("(k p) d -> p k d", p=128))
            wv_sb = const.tile([128, 4, 256], bf16)
            nc.gpsimd.dma_start(wv_sb[:], wvt.ap().rearrange("(k p) d -> p k d", p=128))
            wo_sb = const.tile([128, 2, D], bf16)
            nc.gpsimd.dma_start(wo_sb[:], wot.ap().rearrange("(t p) e -> p t e", p=128))

            eT = big.tile([128, 4, LKV], bf16)
            e_sb = []
            for g in range(4):
                sb = esbp.tile([128, 8, 512], bf16, tag="e_sb", name=f"e_sb{g}")
                nc.gpsimd.dma_start(
                    sb[:],
                    enc.ap()[1024 * g : 1024 * (g + 1), :].rearrange(
                        "(t p) e -> p t e", p=128
                    ),
                )
                e_sb.append(sb)
            for g in range(4):
                for t in range(8):
                    kt = 8 * g + t
                    nc.sync.dma_start(
                        eT[:, :, 128 * kt : 128 * kt + 128],
                        e_sb[g][:, t, :],
                        transpose=True,
                    )

            # --- phase 1: projections ---
            qT = [big.tile([128, LQ], bf16, tag=f"qT{h}", name=f"qT{h}") for h in range(2)]
            kT = [big.tile([128, LKV], bf16, tag=f"kT{h}", name=f"kT{h}") for h in range(2)]
            v_g = [big.tile([128, 8, 256], bf16, tag=f"v{g}", name=f"v{g}") for g in range(4)]

            def proj_head(t):
                # qT[t]: [128 d, 1024 q]
                for c in range(2):
                    ps = ps_m.tile([128, 512], f32, name=f"q_ps{t}{c}", tag="ps")
                    for k in range(4):
                        nc.tensor.matmul(
                            ps[:],
                            wq_sb[:, k, 128 * t : 128 * t + 128],
                            xT[:, k, 512 * c : 512 * c + 512],
                            start=(k == 0),
                            stop=(k == 3),
                        )
                    nc.vector.tensor_scalar_add(
                        qT[t][:, 512 * c : 512 * c + 512], ps[:], bqsb[:, t : t + 1]
                    )
                # kT[t]: [128 d, 4096 kv]
                for c in range(8):
                    ps = ps_m.tile([128, 512], f32, name=f"k_ps{t}{c}", tag="ps")
                    for k in range(4):
                        nc.tensor.matmul(
                            ps[:],
                            wk_sb[:, k, 128 * t : 128 * t + 128],
                            eT[:, k, 512 * c : 512 * c + 512],
                            start=(k == 0),
                            stop=(k == 3),
                        )
                    nc.vector.tensor_scalar_add(
                        kT[t][:, 512 * c : 512 * c + 512], ps[:], bksb[:, t : t + 1]
                    )

            proj_head(0)
            # v: [kv, 256] per kv-tile; both heads at once (N=256)
            for kt in range(32):
                ps = ps_m.tile([128, 256], f32, name=f"v_ps{kt}", tag="ps")
                for k in range(4):
                    nc.tensor.matmul(
                        ps[:],
                        eT[:, k, 128 * kt : 128 * kt + 128],
                        wv_sb[:, k, :],
                        start=(k == 0),
                        stop=(k == 3),
                    )
                nc.scalar.copy(v_g[kt // 8][:, kt % 8, :], ps[:])
            proj_head(1)

            # --- phase 2: attention per head ---
            ctxT = big.tile([128, 2, LQ], bf16)
            for h in range(2):
                ps_ctx = ps_c.tile([128, LQ], f32, name=f"ctx{h}", tag="ctx")
                acc = accp.tile([128, LQ], f32, name=f"acc{h}", tag="acc")
                for kt in range(32):
                    ps_sc = ps_s.tile([128, LQ], f32, name=f"sc{h}_{kt}", tag="sc")
                    lk = kT[h][:, 128 * kt : 128 * kt + 128]
                    for c in range(2):
                        nc.tensor.matmul(
                            ps_sc[:, 512 * c : 512 * c + 512],
                            lk,
                            qT[h][:, 512 * c : 512 * c + 512],
                            start=True,
                            stop=True,
                        )
                    et = expp.tile([128, LQ], bf16, name=f"et{h}_{kt}", tag="et")
                    nc.scalar.activation(et[:], ps_sc[:], EXP, scale=SCALE)
                    lv = v_g[kt // 8][:, kt % 8, 128 * h : 128 * h + 128]
                    for c in range(2):
                        nc.tensor.matmul(
                            ps_ctx[:, 512 * c : 512 * c + 512],
                            lv,
                            et[:, 512 * c : 512 * c + 512],
                            start=(kt == 0),
                            stop=(kt == 31),
                        )
                    if kt == 0:
                        nc.vector.tensor_copy(acc[:], et[:])
                    else:
                        nc.vector.tensor_add(acc[:], acc[:], et[:])
                # denominators: colsum(ACC) via ones-stationary matmul (fp32)
                dn = [ps_m.tile([128, 512], f32, name=f"dn{h}_{i}", tag="ps") for i in range(2)]
                for i in range(2):
                    nc.tensor.matmul(
                        dn[i][0:1, :],
                        ones[:],
                        acc[:, 512 * i : 512 * i + 512],
                        start=True,
                        stop=True,
                    )
                rst = rcp.tile([128, LQ], f32, name=f"rst{h}", tag="rst")
                for i in range(2):
                    nc.vector.reciprocal(
                        rst[0:1, 512 * i : 512 * i + 512], dn[i][0:1, :]
                    )
                rb = rcp.tile([128, LQ], f32, name=f"rb{h}", tag="rb")
                nc.gpsimd.partition_broadcast(rb[:], rst[0:1, :])
                # normalized ctxT (bf16) in one fused PSUM-read multiply
                nc.vector.tensor_mul(ctxT[:, h, :], ps_ctx[:], rb[:])

            # --- phase 3: out-projection + cvec ---
            for j in range(8):
                po = ps_m.tile([128, 512], f32, name=f"o_ps{j}", tag="ps")
                for t in range(2):
                    nc.tensor.matmul(
                        po[:],
                        ctxT[:, t, 128 * j : 128 * j + 128],
                        wo_sb[:, t, :],
                        start=(t == 0),
                        stop=(t == 1),
                    )
                ob = osb.tile([128, 512], f32, name=f"ob{j}", tag="ob")
                nc.vector.tensor_add(ob[:], po[:], cvsb[:])
                nc.sync.dma_start(
                    outp.ap().rearrange("(j p) e -> p j e", p=128)[:, j, :], ob[:]
                )

    nc.compile()
    return nc


def _get_nc():
    if "nc" not in _compiled:
        _compiled["nc"] = _build()
    return _compiled["nc"]


def _make_in_maps(x, encoder_feats, Wq, Wk, Wv, bq, bk, bv, Wo, bo):
    f = np.float32
    x = np.asarray(x, f)
    encoder_feats = np.asarray(encoder_feats, f)
    Wq, Wk, Wv, Wo = (np.asarray(a, f) for a in (Wq, Wk, Wv, Wo))
    bq, bk, bv, bo = (np.asarray(a, f) for a in (bq, bk, bv, bo))
    in_maps = []
    for c in range(NCORES):
        b, hp = c // 2, c % 2
        sl = slice(256 * hp, 256 * hp + 256)
        cv = Wo[:, sl] @ bv[sl]
        if hp == 0:
            cv = cv + bo
        in_maps.append(
            {
                "xb": x[b],
                "enc": encoder_feats[b],
                "wqt": np.ascontiguousarray(Wq[sl, :].T),
                "wkt": np.ascontiguousarray(Wk[sl, :].T),
                "wvt": np.ascontiguousarray(Wv[sl, :].T),
                "wot": np.ascontiguousarray(Wo[:, sl].T),
                "bq2": np.ascontiguousarray(bq[sl].reshape(2, 128).T),
                "bk2": np.ascontiguousarray(bk[sl].reshape(2, 128).T),
                "cvec": np.ascontiguousarray(cv, dtype=f),
            }
        )
    return in_maps


def kernel(x, encoder_feats, Wq, Wk, Wv, bq, bk, bv, Wo, bo, _trace=False):
    from concourse.bass_utils import run_bass_kernel_spmd

    nc = _get_nc()
    in_maps = _make_in_maps(x, encoder_feats, Wq, Wk, Wv, bq, bk, bv, Wo, bo)
    kw = {}
    if _trace:
        kw = dict(trace=True, trace_cores=[0])
    res = run_bass_kernel_spmd(nc, in_maps, core_ids=list(range(NCORES)), **kw)
    _compiled["last_res"] = res
    out = np.empty((B, LQ, D), np.float32)
    for b in range(B):
        out[b] = res.results[2 * b]["outp"] + res.results[2 * b + 1]["outp"]
    return out


# revision 5
# speedup vs baseline: 1.1714x; 1.1714x over previous
"""Trainium2 Bass kernel for nn_AttentionFusion (cross-attention, B=4, LQ=1024,
LKV=4096, D=512, H=4 heads of 128).

Sharding: 8 cores = (batch b in 0..3) x (head-pair hp in 0..1). Core c = 2*b+hp
computes attention for heads {2hp, 2hp+1} of batch b plus its partial
out-projection (tensor-parallel split of Wo). Host sums the two partials per
batch (the TP un-shard); everything else runs on device in bf16 with fp32
accumulation.

Per-core dataflow:
  xT [e,q], eT [e,kv]  <- gpsimd cast-load (f32->bf16) + HWDGE xbar transpose
  qT/kT [d,*]          <- weight-stationary projections; per-partition bias
                          fused into the PSUM->SBUF copy on ACT
  v [kv,d]             <- encoder-stationary projection (bv folded into cvec:
                          softmax rows sum to 1, so attn@(v0+bv)=attn@v0+bv)
  scoresT [kv,q] (PSUM) = kT-tile.T @ qT ; P = exp(scale*scoresT) on ACT (bf16)
  ctx~T [d,q]  (PSUM)  += v-tile.T @ P  over kv tiles (unnormalized)
  denom: bf16 pairwise tree of P tiles on DVE -> f32 -> PE-transpose ->
         free-dim reduce -> reciprocal (per-partition [q,1] layout)
  out[q,e] = (ctx~T.T @ Wo_sl.T) * recip[q]  + cvec  -> DRAM f32
"""

import numpy as np

B, LQ, LKV, D, H, HD = 4, 1024, 4096, 512, 4, 128
NCORES = 8
SCALE = 1.0 / float(np.sqrt(HD))

_compiled = {}


def _build():
    import concourse.bacc as bacc
    import concourse.mybir as mybir
    from concourse import tile
    from concourse.masks import make_identity

    bf16, f32 = mybir.dt.bfloat16, mybir.dt.float32
    EXP = mybir.ActivationFunctionType.Exp
    IDN = mybir.ActivationFunctionType.Identity

    nc = bacc.Bacc(
        "TRN2",
        target_bir_lowering=False,
        debug=False,
        enable_asserts=True,
        num_devices=NCORES,
    )

    xb = nc.dram_tensor("xb", [LQ, D], f32, kind="ExternalInput")
    enc = nc.dram_tensor("enc", [LKV, D], f32, kind="ExternalInput")
    wqt = nc.dram_tensor("wqt", [D, 256], f32, kind="ExternalInput")  # Wq[sl].T
    wkt = nc.dram_tensor("wkt", [D, 256], f32, kind="ExternalInput")
    wvt = nc.dram_tensor("wvt", [D, 256], f32, kind="ExternalInput")
    wot = nc.dram_tensor("wot", [256, D], f32, kind="ExternalInput")  # Wo[:,sl].T
    bq2 = nc.dram_tensor("bq2", [128, 2], f32, kind="ExternalInput")
    bk2 = nc.dram_tensor("bk2", [128, 2], f32, kind="ExternalInput")
    cvec = nc.dram_tensor("cvec", [D], f32, kind="ExternalInput")
    outp = nc.dram_tensor("outp", [LQ, D], f32, kind="ExternalOutput")

    with tile.TileContext(nc) as tc:
        with (
            tc.tile_pool(name="const", bufs=1) as const,
            tc.tile_pool(name="big", bufs=1) as big,
            tc.tile_pool(name="expp", bufs=4) as expp,
            tc.tile_pool(name="tree", bufs=7) as treep,
            tc.tile_pool(name="accp", bufs=2) as accp,
            tc.tile_pool(name="smal", bufs=4) as smal,
            tc.tile_pool(name="osb", bufs=4) as osb,
            tc.tile_pool(name="ps", bufs=3, space="PSUM") as psp,
            tc.tile_pool(name="ps_c", bufs=1, space="PSUM") as ps_c,
        ):
            # --- constants ---
            ones = const.tile([128, 1], f32)
            nc.vector.memset(ones[:], 1.0)
            ident = const.tile([128, 128], f32)
            make_identity(nc, ident[:])
            bqsb = const.tile([128, 2], f32)
            nc.sync.dma_start(bqsb[:], bq2[:])
            bksb = const.tile([128, 2], f32)
            nc.sync.dma_start(bksb[:], bk2[:])
            cvst = const.tile([128, D], f32)
            nc.sync.dma_start(cvst[0:1, :], cvec.ap().unsqueeze(0))
            cvsb = const.tile([128, D], f32)
            nc.gpsimd.partition_broadcast(cvsb[:], cvst[0:1, :])
            # warm the ACT exp table set early (~2.7us table load)
            warm = const.tile([128, 1], f32)
            nc.scalar.activation(warm[:], ones[:], EXP)

            # --- phase 0: loads (cast f32->bf16 on SWDGE) + xbar transposes ---
            x_sb = big.tile([128, 8, 512], bf16)
            nc.gpsimd.dma_start(x_sb[:], xb.ap().rearrange("(t p) e -> p t e", p=128))
            xT = big.tile([128, 4, LQ], bf16)
            for t in range(8):
                nc.sync.dma_start(
                    xT[:, :, 128 * t : 128 * t + 128], x_sb[:, t, :], transpose=True
                )

            wq_sb = const.tile([128, 4, 256], bf16)
            nc.gpsimd.dma_start(wq_sb[:], wqt.ap().rearrange("(k p) d -> p k d", p=128))
            wk_sb = const.tile([128, 4, 256], bf16)
            nc.gpsimd.dma_start(wk_sb[:], wkt.ap().rearrange("(k p) d -> p k d", p=128))
            wv_sb = const.tile([128, 4, 256], bf16)
            nc.gpsimd.dma_start(wv_sb[:], wvt.ap().rearrange("(k p) d -> p k d", p=128))
            wo_sb = const.tile([128, 2, D], bf16)
            nc.gpsimd.dma_start(wo_sb[:], wot.ap().rearrange("(t p) e -> p t e", p=128))

            # encoder in 4 groups of 1024 kv rows; eT per group
            eT = []
            for g in range(4):
                sb = big.tile([128, 8, 512], bf16, tag="e_sb", name=f"e_sb{g}")
                nc.gpsimd.dma_start(
                    sb[:],
                    enc.ap()[1024 * g : 1024 * (g + 1), :].rearrange(
                        "(t p) e -> p t e", p=128
                    ),
                )
                eTg = big.tile([128, 4, 1024], bf16, tag=f"eT{g}", name=f"eT{g}")
                for t in range(8):
                    eng = nc.sync if t % 2 == 0 else nc.scalar
                    eng.dma_start(
                        eTg[:, :, 128 * t : 128 * t + 128], sb[:, t, :], transpose=True
                    )
                eT.append(eTg)

            def eT_ap(k, kv0, kv1):
                """eT[:, k, kv0:kv1] across group tiles; must stay in one group."""
                g = kv0 // 1024
                assert kv1 <= 1024 * (g + 1)
                return eT[g][:, k, kv0 - 1024 * g : kv1 - 1024 * g]

            # --- phase 1: projections ---
            qT = [
                big.tile([128, LQ], bf16, tag=f"qT{h}", name=f"qT{h}")
                for h in range(2)
            ]
            kT = [
                big.tile([128, LKV], bf16, tag=f"kT{h}", name=f"kT{h}")
                for h in range(2)
            ]
            v_g = [
                big.tile([128, 8, 256], bf16, tag=f"v{g}", name=f"v{g}")
                for g in range(4)
            ]

            def proj_q(t):
                for c in range(2):
                    ps = psp.tile([128, LQ], f32, name=f"q_ps{t}{c}", tag="sc")
                    for k in range(4):
                        nc.tensor.matmul(
                            ps[:, 0:512],
                            wq_sb[:, k, 128 * t : 128 * t + 128],
                            xT[:, k, 512 * c : 512 * c + 512],
                            start=(k == 0),
                            stop=(k == 3),
                        )
                    nc.scalar.activation(
                        qT[t][:, 512 * c : 512 * c + 512],
                        ps[:, 0:512],
                        IDN,
                        bias=bqsb[:, t : t + 1],
                    )

            def proj_k(t):
                for c in range(8):
                    ps = psp.tile([128, LQ], f32, name=f"k_ps{t}{c}", tag="sc")
                    for k in range(4):
                        nc.tensor.matmul(
                            ps[:, 0:512],
                            wk_sb[:, k, 128 * t : 128 * t + 128],
                            eT_ap(k, 512 * c, 512 * c + 512),
                            start=(k == 0),
                            stop=(k == 3),
                        )
                    nc.scalar.activation(
                        kT[t][:, 512 * c : 512 * c + 512],
                        ps[:, 0:512],
                        IDN,
                        bias=bksb[:, t : t + 1],
                    )

            def proj_v(g):
                for i in range(8):
                    kt = 8 * g + i
                    ps = psp.tile([128, LQ], f32, name=f"v_ps{kt}", tag="sc")
                    for k in range(4):
                        nc.tensor.matmul(
                            ps[:, 0:256],
                            eT_ap(k, 128 * kt, 128 * kt + 128),
                            wv_sb[:, k, :],
                            start=(k == 0),
                            stop=(k == 3),
                        )
                    nc.vector.tensor_copy(v_g[g][:, i, :], ps[:, 0:256])

            proj_q(0)
            proj_k(0)
            for g in range(4):
                proj_v(g)
            proj_q(1)
            proj_k(1)

            # --- phase 2: attention per head (h == d-tile index in our slice) ---
            ctxT = big.tile([128, 2, LQ], bf16)
            recip = []
            for h in range(2):
                ps_ctx = ps_c.tile([128, LQ], f32, name=f"ctx{h}", tag="ctx")
                # binary-counter tree of bf16 partial sums of P tiles
                levels: list = [None] * 6
                for kt in range(32):
                    ps_sc = psp.tile([128, LQ], f32, name=f"sc{h}_{kt}", tag="sc")
                    lk = kT[h][:, 128 * kt : 128 * kt + 128]
                    for c in range(2):
                        nc.tensor.matmul(
                            ps_sc[:, 512 * c : 512 * c + 512],
                            lk,
                            qT[h][:, 512 * c : 512 * c + 512],
                            start=True,
                            stop=True,
                        )
                    et = expp.tile([128, LQ], bf16, name=f"et{h}_{kt}", tag="et")
                    nc.scalar.activation(et[:], ps_sc[:], EXP, scale=SCALE)
                    lv = v_g[kt // 8][:, kt % 8, 128 * h : 128 * h + 128]
                    for c in range(2):
                        nc.tensor.matmul(
                            ps_ctx[:, 512 * c : 512 * c + 512],
                            lv,
                            et[:, 512 * c : 512 * c + 512],
                            start=(kt == 0),
                            stop=(kt == 31),
                        )
                    # push et into the tree (bf16 adds run in DVE 2x mode)
                    cur, lvl = et, 0
                    while levels[lvl] is not None:
                        nxt = treep.tile(
                            [128, LQ], bf16, name=f"tr{h}_{kt}_{lvl}", tag="tr"
                        )
                        nc.vector.tensor_add(nxt[:], levels[lvl][:], cur[:])
                        levels[lvl] = None
                        cur, lvl = nxt, lvl + 1
                    levels[lvl] = cur
                root = levels[5]
                assert root is not None and all(l is None for l in levels[:5])
                # unnormalized ctx~T to SBUF (bf16)
                nc.vector.tensor_copy(ctxT[:, h, :], ps_ctx[:])
                # denominators -> [q-part, 8] via f32 PE transpose + reduce
                acc = accp.tile([128, LQ], f32, name=f"acc{h}", tag="acc")
                nc.vector.tensor_copy(acc[:], root[:])
                den = smal.tile([128, 8], f32, name=f"den{h}", tag="den")
                for half in range(2):
                    pt = psp.tile([128, LQ], f32, name=f"dt{h}{half}", tag="sc")
                    for j in range(4):
                        jj = 4 * half + j
                        nc.tensor.transpose(
                            pt[:, 128 * j : 128 * j + 128],
                            acc[:, 128 * jj : 128 * jj + 128],
                            ident[:],
                        )
                    nc.vector.tensor_reduce(
                        den[:, 4 * half : 4 * half + 4],
                        pt[:, 0:512].rearrange("p (j q) -> p j q", j=4),
                        axis=mybir.AxisListType.X,
                        op=mybir.AluOpType.add,
                    )
                rc = smal.tile([128, 8], f32, name=f"rc{h}", tag="rc")
                nc.vector.reciprocal(rc[:], den[:])
                recip.append(rc)

            # --- phase 3: out-projection, normalize, add cvec ---
            for j in range(8):
                po = []
                for t in range(2):
                    p = psp.tile([128, LQ], f32, name=f"o_ps{t}_{j}", tag="sc")
                    nc.tensor.matmul(
                        p[:, 0:512],
                        ctxT[:, t, 128 * j : 128 * j + 128],
                        wo_sb[:, t, :],
                        start=True,
                        stop=True,
                    )
                    po.append(p)
                # normalize per head on ACT (per-partition scale), sum on DVE
                nrm = []
                for t in range(2):
                    n = osb.tile([128, 512], f32, name=f"nrm{t}_{j}", tag=f"nrm{t}")
                    nc.scalar.activation(
                        n[:], po[t][:, 0:512], IDN, scale=recip[t][:, j : j + 1]
                    )
                    nrm.append(n)
                ob = osb.tile([128, 512], f32, name=f"ob{j}", tag="ob")
                nc.vector.tensor_add(ob[:], nrm[0][:], nrm[1][:])
                nc.vector.tensor_add(ob[:], ob[:], cvsb[:])
                nc.sync.dma_start(
                    outp.ap().rearrange("(j p) e -> p j e", p=128)[:, j, :], ob[:]
                )

    nc.compile()
    return nc


def _get_nc():
    if "nc" not in _compiled:
        _compiled["nc"] = _build()
    return _compiled["nc"]


def _make_in_maps(x, encoder_feats, Wq, Wk, Wv, bq, bk, bv, Wo, bo):
    f = np.float32
    x = np.asarray(x, f)
    encoder_feats = np.asarray(encoder_feats, f)
    Wq, Wk, Wv, Wo = (np.asarray(a, f) for a in (Wq, Wk, Wv, Wo))
    bq, bk, bv, bo = (np.asarray(a, f) for a in (bq, bk, bv, bo))
    in_maps = []
    for c in range(NCORES):
        b, hp = c // 2, c % 2
        sl = slice(256 * hp, 256 * hp + 256)
        cv = Wo[:, sl] @ bv[sl]
        if hp == 0:
            cv = cv + bo
        in_maps.append(
            {
                "xb": x[b],
                "enc": encoder_feats[b],
                "wqt": np.ascontiguousarray(Wq[sl, :].T),
                "wkt": np.ascontiguousarray(Wk[sl, :].T),
                "wvt": np.ascontiguousarray(Wv[sl, :].T),
                "wot": np.ascontiguousarray(Wo[:, sl].T),
                "bq2": np.ascontiguousarray(bq[sl].reshape(2, 128).T),
                "bk2": np.ascontiguousarray(bk[sl].reshape(2, 128).T),
                "cvec": np.ascontiguousarray(cv, dtype=f),
            }
        )
    return in_maps


def kernel(x, encoder_feats, Wq, Wk, Wv, bq, bk, bv, Wo, bo, _trace=False):
    from concourse.bass_utils import run_bass_kernel_spmd

    nc = _get_nc()
    in_maps = _make_in_maps(x, encoder_feats, Wq, Wk, Wv, bq, bk, bv, Wo, bo)
    kw = {}
    if _trace:
        kw = dict(trace=True, trace_cores=[0])
    res = run_bass_kernel_spmd(nc, in_maps, core_ids=list(range(NCORES)), **kw)
    _compiled["last_res"] = res
    out = np.empty((B, LQ, D), np.float32)
    for b in range(B):
        out[b] = res.results[2 * b]["outp"] + res.results[2 * b + 1]["outp"]
    return out


# revision 13
# speedup vs baseline: 1.4908x; 1.2726x over previous
"""Trainium2 Bass kernel for nn_AttentionFusion (cross-attention, B=4, LQ=1024,
LKV=4096, D=512, H=4 heads of 128).

Sharding: 8 cores = (batch b in 0..3) x (head-pair hp in 0..1). Core c = 2*b+hp
computes attention for heads {2hp, 2hp+1} of batch b plus its partial
out-projection (tensor-parallel split of Wo). Host sums the two partials per
batch (the TP un-shard); everything else runs on device in bf16 with fp32
accumulation.

Layout trick: rows are loaded p-major ("(p t) e -> p t e") so every partition
reads one contiguous 16KB block (fast DMA). This permutes the kv order, which
attention is invariant to (kT / v / P all share the ordering), and permutes q,
which is undone for free in the output DMA's DRAM access pattern.

Per-core dataflow:
  xT [e,q], eT [e,kv]  <- gpsimd cast-load (f32->bf16) + HWDGE xbar transpose
  qT/kT [d,*]          <- weight-stationary projections; per-partition bias
                          fused into the PSUM->SBUF copy on ACT
  v [kv,d]             <- encoder-stationary projection (bv folded into cvec:
                          softmax rows sum to 1, so attn@(v0+bv)=attn@v0+bv)
  scoresT [kv,q] (PSUM) = kT-tile.T @ qT ; P = exp(scale*scoresT) on ACT (bf16)
  ctx~T [d,q]  (PSUM)  += v-tile.T @ P  over kv tiles (unnormalized)
  denom: bf16 pairwise tree of P tiles on DVE -> f32 -> PE-transpose ->
         free-dim reduce -> reciprocal (per-partition [q,1] layout)
  out[q,e] = (ctx~T.T @ Wo_sl.T) * recip[q]  + cvec  -> DRAM f32
"""

import numpy as np

B, LQ, LKV, D, H, HD = 4, 1024, 4096, 512, 4, 128
NCORES = 8
SCALE = 1.0 / float(np.sqrt(HD))

_compiled = {}


def _build():
    import concourse.bacc as bacc
    import concourse.mybir as mybir
    from concourse import tile
    from concourse.masks import make_identity

    bf16, f32 = mybir.dt.bfloat16, mybir.dt.float32
    EXP = mybir.ActivationFunctionType.Exp
    IDN = mybir.ActivationFunctionType.Identity

    nc = bacc.Bacc(
        "TRN2",
        target_bir_lowering=False,
        debug=False,
        enable_asserts=True,
        num_devices=NCORES,
    )

    xb = nc.dram_tensor("xb", [LQ, D], f32, kind="ExternalInput")
    enc = nc.dram_tensor("enc", [LKV, D], f32, kind="ExternalInput")
    wqt = nc.dram_tensor("wqt", [128, 1024], f32, kind="ExternalInput")
    wkt = nc.dram_tensor("wkt", [128, 1024], f32, kind="ExternalInput")
    wvt = nc.dram_tensor("wvt", [128, 1024], f32, kind="ExternalInput")
    wot = nc.dram_tensor("wot", [128, 1024], f32, kind="ExternalInput")
    bq2 = nc.dram_tensor("bq2", [128, 2], f32, kind="ExternalInput")
    bk2 = nc.dram_tensor("bk2", [128, 2], f32, kind="ExternalInput")
    cvec = nc.dram_tensor("cvec", [D], f32, kind="ExternalInput")
    outp = nc.dram_tensor("outp", [LQ, D], f32, kind="ExternalOutput")

    with tile.TileContext(nc) as tc:
        with (
            tc.tile_pool(name="const", bufs=1) as const,
            tc.tile_pool(name="big", bufs=1) as big,
            tc.tile_pool(name="expp", bufs=4) as expp,
            tc.tile_pool(name="tree", bufs=7) as treep,
            tc.tile_pool(name="accp", bufs=2) as accp,
            tc.tile_pool(name="smal", bufs=4) as smal,
            tc.tile_pool(name="nrm0p", bufs=8) as nrm0p,
            tc.tile_pool(name="osb", bufs=4) as osb,
            tc.tile_pool(name="wstp", bufs=2) as wstp,
            tc.tile_pool(name="ps", bufs=3, space="PSUM") as psp,
            tc.tile_pool(name="ps_c", bufs=1, space="PSUM") as ps_c,
        ):
            # --- constants ---
            ones = const.tile([128, 1], f32)
            nc.vector.memset(ones[:], 1.0)
            ident = const.tile([128, 128], f32)
            make_identity(nc, ident[:])
            identb = const.tile([128, 128], bf16)
            make_identity(nc, identb[:])
            bqsb = const.tile([128, 2], f32)
            nc.sync.dma_start(bqsb[:], bq2[:])
            bksb = const.tile([128, 2], f32)
            nc.sync.dma_start(bksb[:], bk2[:])
            # warm the ACT exp table set early (~2.7us table load)
            warm = const.tile([128, 1], f32)
            nc.scalar.activation(warm[:], ones[:], EXP)

            # --- phase 0+1 interleaved: loads, transposes, projections ---
            # first encoder group load goes first (longest pole)
            e_sbs = [
                big.tile([128, 8, 512], bf16, tag="e_sb", name=f"e_sb{g}")
                for g in range(4)
            ]
            nc.gpsimd.dma_start(
                e_sbs[0][:], enc.ap()[0:1024, :].rearrange("(p t) e -> p t e", t=8)
            )
            # x: partition p holds rows 8p..8p+7 (contiguous 16KB reads)
            x_sb = big.tile([128, 8, 512], bf16)
            nc.gpsimd.dma_start(x_sb[:], xb.ap().rearrange("(p t) e -> p t e", t=8))
            xT = big.tile([128, 4, LQ], bf16)
            for t in range(8):
                pt = psp.tile([128, 512], bf16, name=f"xt_ps{t}", tag="sc")
                for j in range(4):
                    nc.tensor.transpose(
                        pt[:, 128 * j : 128 * j + 128],
                        x_sb[:, t, 128 * j : 128 * j + 128],
                        identb[:],
                    )
                nc.vector.tensor_copy(
                    xT[:, :, 128 * t : 128 * t + 128],
                    pt[:].rearrange("p (j q) -> p j q", j=4),
                )

            wk_sb = const.tile([128, 4, 256], bf16)
            wv_sb = const.tile([128, 4, 256], bf16)
            wq_sb = const.tile([128, 4, 256], bf16)
            wo_sb = const.tile([128, 2, D], bf16)
            for wdram, wsb, nk in (
                (wkt, wk_sb, 4),
                (wvt, wv_sb, 4),
                (wqt, wq_sb, 4),
                (wot, wo_sb, 2),
            ):
                wst = wstp.tile([128, 1024], f32, tag="wst", name=f"wst_{wdram.name}")
                nc.sync.dma_start(wst[:], wdram[:])
                nc.vector.tensor_copy(
                    wsb[:], wst[:].rearrange("p (k d) -> p k d", k=nk)
                )

            qT = [
                big.tile([128, LQ], bf16, tag=f"qT{h}", name=f"qT{h}")
                for h in range(2)
            ]
            # kT per (head, kv-group of 1024)
            kT = [
                [
                    big.tile([128, 1024], bf16, tag=f"kT{h}_{g}", name=f"kT{h}_{g}")
                    for g in range(4)
                ]
                for h in range(2)
            ]
            v_g = [
                big.tile([128, 8, 256], bf16, tag=f"v{g}", name=f"v{g}")
                for g in range(4)
            ]

            def proj_q(t):
                for c in range(2):
                    ps = psp.tile([128, LQ], f32, name=f"q_ps{t}{c}", tag="sc")
                    for k in range(4):
                        nc.tensor.matmul(
                            ps[:, 0:512],
                            wq_sb[:, k, 128 * t : 128 * t + 128],
                            xT[:, k, 512 * c : 512 * c + 512],
                            start=(k == 0),
                            stop=(k == 3),
                        )
                    nc.scalar.activation(
                        qT[t][:, 512 * c : 512 * c + 512],
                        ps[:, 0:512],
                        IDN,
                        bias=bqsb[:, t : t + 1],
                    )

            # encoder groups: load -> transpose -> k-proj h0 -> v-proj
            eT = [None] * 4
            proj_k_ref = {}

            def proj_k(h, g):
                return proj_k_ref["f"](h, g)

            def enc_group(g):
                sb = e_sbs[g]
                if g > 0:
                    nc.gpsimd.dma_start(
                        sb[:],
                        enc.ap()[1024 * g : 1024 * (g + 1), :].rearrange(
                            "(p t) e -> p t e", t=8
                        ),
                    )
                eTg = big.tile([128, 4, 1024], bf16, tag=f"eT{g}", name=f"eT{g}")
                for t in range(8):
                    pt = psp.tile([128, 512], bf16, name=f"et_ps{g}{t}", tag="sc")
                    for j in range(4):
                        nc.tensor.transpose(
                            pt[:, 128 * j : 128 * j + 128],
                            sb[:, t, 128 * j : 128 * j + 128],
                            identb[:],
                        )
                    dst = eTg[:, :, 128 * t : 128 * t + 128]
                    src = pt[:].rearrange("p (j q) -> p j q", j=4)
                    if t % 2 == 0:
                        nc.vector.tensor_copy(dst, src)
                    else:
                        nc.scalar.copy(dst, src)
                eT[g] = eTg
                if g == 0:
                    proj_q(0)
                    proj_q(1)
                proj_k(0, g)
                for i in range(8):
                    ps = psp.tile([128, LQ], f32, name=f"v_ps{g}{i}", tag="sc")
                    for k in range(4):
                        nc.tensor.matmul(
                            ps[:, 0:256],
                            eTg[:, k, 128 * i : 128 * i + 128],
                            wv_sb[:, k, :],
                            start=(k == 0),
                            stop=(k == 3),
                        )
                    nc.vector.tensor_copy(v_g[g][:, i, :], ps[:, 0:256])

            # --- phase 2: attention, software-pipelined with group chains ---
            ctxT = big.tile([128, 2, LQ], bf16)
            recip = []
            nrm0 = []
            att_state = {}

            def attn_segment(h, g, inject=None):
                if g == 0:
                    att_state[h] = {"ps_ctx": ps_c.tile([128, LQ], f32, name=f"ctx{h}", tag="ctx"), "levels": [None] * 6}
                st = att_state[h]
                ps_ctx, levels = st["ps_ctx"], st["levels"]
                for kt in range(8 * g, 8 * g + 8):
                    ps_sc = psp.tile([128, LQ], f32, name=f"sc{h}_{kt}", tag="sc")
                    lk = kT[h][kt // 8][:, 128 * (kt % 8) : 128 * (kt % 8) + 128]
                    for c in range(2):
                        nc.tensor.matmul(
                            ps_sc[:, 512 * c : 512 * c + 512],
                            lk,
                            qT[h][:, 512 * c : 512 * c + 512],
                            start=True,
                            stop=True,
                        )
                    et = expp.tile([128, LQ], bf16, name=f"et{h}_{kt}", tag="et")
                    nc.scalar.activation(et[:], ps_sc[:], EXP, scale=SCALE)
                    lv = v_g[kt // 8][:, kt % 8, 128 * h : 128 * h + 128]
                    for c in range(2):
                        nc.tensor.matmul(
                            ps_ctx[:, 512 * c : 512 * c + 512],
                            lv,
                            et[:, 512 * c : 512 * c + 512],
                            start=(kt == 0),
                            stop=(kt == 31),
                        )
                    cur, lvl = et, 0
                    while levels[lvl] is not None:
                        nxt = treep.tile(
                            [128, LQ], bf16, name=f"tr{h}_{kt}_{lvl}", tag="tr"
                        )
                        nc.vector.tensor_add(nxt[:], levels[lvl][:], cur[:])
                        levels[lvl] = None
                        cur, lvl = nxt, lvl + 1
                    levels[lvl] = cur
                    if kt % 8 == 6 and inject is not None:
                        inject()

            def attn_finish(h):
                st = att_state[h]
                root = st["levels"][5]
                assert root is not None
                nc.vector.tensor_copy(ctxT[:, h, :], st["ps_ctx"][:])
                acc = accp.tile([128, LQ], f32, name=f"acc{h}", tag="acc")
                nc.vector.tensor_copy(acc[:], root[:])
                den = smal.tile([128, 8], f32, name=f"den{h}", tag="den")
                for half in range(2):
                    pt = psp.tile([128, LQ], f32, name=f"dt{h}{half}", tag="sc")
                    for j in range(4):
                        jj = 4 * half + j
                        nc.tensor.transpose(
                            pt[:, 128 * j : 128 * j + 128],
                            acc[:, 128 * jj : 128 * jj + 128],
                            ident[:],
                        )
                    nc.vector.tensor_reduce(
                        den[:, 4 * half : 4 * half + 4],
                        pt[:, 0:512].rearrange("p (j q) -> p j q", j=4),
                        axis=mybir.AxisListType.X,
                        op=mybir.AluOpType.add,
                    )
                rc = smal.tile([128, 8], f32, name=f"rc{h}", tag="rc")
                nc.vector.reciprocal(rc[:], den[:])
                recip.append(rc)

            def outproj_h0():
                attn_finish(0)
                for j in range(8):
                    p = psp.tile([128, LQ], f32, name=f"o_ps0_{j}", tag="sc")
                    nc.tensor.matmul(
                        p[:, 0:512],
                        ctxT[:, 0, 128 * j : 128 * j + 128],
                        wo_sb[:, 0, :],
                        start=True,
                        stop=True,
                    )
                    n = nrm0p.tile([128, 512], f32, name=f"nrm0_{j}", tag="nrm0")
                    nc.vector.tensor_scalar_mul(n[:], p[:, 0:512], recip[0][:, j : j + 1])
                    nrm0.append(n)

            def _proj_k(h, g):
                for c in range(2):  # kv chunks of 512 within the group
                    ps = psp.tile([128, LQ], f32, name=f"k_ps{h}{g}{c}", tag="sc")
                    for k in range(4):
                        nc.tensor.matmul(
                            ps[:, 0:512],
                            wk_sb[:, k, 128 * h : 128 * h + 128],
                            eT[g][:, k, 512 * c : 512 * c + 512],
                            start=(k == 0),
                            stop=(k == 3),
                        )
                    nc.scalar.activation(
                        kT[h][g][:, 512 * c : 512 * c + 512],
                        ps[:, 0:512],
                        IDN,
                        bias=bksb[:, h : h + 1],
                    )

            proj_k_ref["f"] = _proj_k

            # group chains up front (PE in-order: projections before attention),
            # then attention h0, h0 epilogue, attention h1
            for g in range(4):
                enc_group(g)
                proj_k(1, g)
            for g in range(4):
                attn_segment(0, g)

            # cvec broadcast (needed only at the very end)
            cvst = const.tile([128, D], f32)
            nc.sync.dma_start(cvst[0:1, :], cvec.ap().unsqueeze(0))
            cvsb = const.tile([128, D], f32)
            nc.gpsimd.partition_broadcast(cvsb[:], cvst[0:1, :])

            outproj_h0()
            for g in range(4):
                attn_segment(1, g)
            attn_finish(1)

            # head 1 out-projection + combine + store (q un-permute in DRAM AP)
            out_ap = outp.ap().rearrange("(p t) e -> p t e", t=8)
            for j in range(8):
                p = psp.tile([128, LQ], f32, name=f"o_ps1_{j}", tag="sc")
                nc.tensor.matmul(
                    p[:, 0:512],
                    ctxT[:, 1, 128 * j : 128 * j + 128],
                    wo_sb[:, 1, :],
                    start=True,
                    stop=True,
                )
                n1 = osb.tile([128, 512], f32, name=f"nrm1_{j}", tag="nrm1")
                nc.scalar.activation(
                    n1[:], p[:, 0:512], IDN, scale=recip[1][:, j : j + 1]
                )
                ob = osb.tile([128, 512], f32, name=f"ob{j}", tag="ob")
                nc.vector.tensor_add(ob[:], nrm0[j][:], n1[:])
                nc.vector.tensor_add(ob[:], ob[:], cvsb[:])
                nc.sync.dma_start(out_ap[:, j, :], ob[:])

    nc.compile()
    return nc


def _get_nc():
    if "nc" not in _compiled:
        _compiled["nc"] = _build()
    return _compiled["nc"]


def _warr(wt, k):
    """[k*128, n] -> [128, k*n] so partition p reads one contiguous block."""
    n = wt.shape[1]
    return np.ascontiguousarray(
        wt.reshape(k, 128, n).transpose(1, 0, 2).reshape(128, k * n)
    )


def _make_in_maps(x, encoder_feats, Wq, Wk, Wv, bq, bk, bv, Wo, bo):
    f = np.float32
    x = np.asarray(x, f)
    encoder_feats = np.asarray(encoder_feats, f)
    Wq, Wk, Wv, Wo = (np.asarray(a, f) for a in (Wq, Wk, Wv, Wo))
    bq, bk, bv, bo = (np.asarray(a, f) for a in (bq, bk, bv, bo))
    in_maps = []
    for c in range(NCORES):
        b, hp = c // 2, c % 2
        sl = slice(256 * hp, 256 * hp + 256)
        cv = Wo[:, sl] @ bv[sl]
        if hp == 0:
            cv = cv + bo
        in_maps.append(
            {
                "xb": x[b],
                "enc": encoder_feats[b],
                "wqt": _warr(Wq[sl, :].T, 4),
                "wkt": _warr(Wk[sl, :].T, 4),
                "wvt": _warr(Wv[sl, :].T, 4),
                "wot": _warr(Wo[:, sl].T, 2),
                "bq2": np.ascontiguousarray(bq[sl].reshape(2, 128).T),
                "bk2": np.ascontiguousarray(bk[sl].reshape(2, 128).T),
                "cvec": np.ascontiguousarray(cv, dtype=f),
            }
        )
    return in_maps


def kernel(x, encoder_feats, Wq, Wk, Wv, bq, bk, bv, Wo, bo, _trace=False):
    from concourse.bass_utils import run_bass_kernel_spmd

    nc = _get_nc()
    in_maps = _make_in_maps(x, encoder_feats, Wq, Wk, Wv, bq, bk, bv, Wo, bo)
    kw = {}
    if _trace:
        kw = dict(trace=True, trace_cores=[0])
    res = run_bass_kernel_spmd(nc, in_maps, core_ids=list(range(NCORES)), **kw)
    _compiled["last_res"] = res
    out = np.empty((B, LQ, D), np.float32)
    for b in range(B):
        out[b] = res.results[2 * b]["outp"] + res.results[2 * b + 1]["outp"]
    return out
